# revision 1
# baseline (speedup 1.0000x reference)
"""GCN layer kernel for nn_GCNLayer_20547123544324 on 8 Trainium2 NeuronCores.

Computes a PyG-style GCNConv:
    out = D^-1/2 (A + I) D^-1/2 (x @ W) + b
       == (D^-1/2 (A + I) D^-1/2 x) @ W + b        (associativity)

Node-partitioned (per the sharding hint): 12500 dst nodes per core; edges
(incl. self-loops) bucketed by 128-node dst windows, padded to a static
B=19 blocks of 128 edges per window (uniform SPMD program).

Device pipeline per core:
  - per 128-edge block: indirect-DMA row gather x[src] (bf16 rows padded
    to 256B stride; one row per partition — the canonical form this
    hardware's SWDGE path implements correctly)
  - per window: one-hot scatter matrices S'[e, j] = norm_e * (dst_e == j)
    built by DVE iota-compare then norm scale (norm folded here keeps the
    per-block DVE work off the critical path)
  - PE: aggT[64 feat, 128 dst] += gathered_block.T @ S'_block, accumulated
    in PSUM over the window's 19 blocks
  - PE: out_win[128, 64] = aggT.T @ W  (aggT is already in lhsT layout)
  - DVE bias add, DMA out.

Self-contained: hardcoded N=100000, E=1600000, D=64, 8 cores.
"""
import numpy as np
import ml_dtypes

import jax
from jax.sharding import Mesh, PartitionSpec
from jax.experimental.shard_map import shard_map

import concourse.bass as bass
import concourse.mybir as mybir
import concourse.tile as tile
from concourse import bacc
from concourse.bass2jax import _bass_exec_p, install_neuronx_cc_hook, \
    partition_id_tensor

N = 100000
E = 1600000
D = 64
M = 8                 # cores
NPC = N // M          # 12500 nodes per core
P = 128
NWIN = -(-NPC // P)   # 98 windows (last holds 84 nodes)
B = 19                # blocks of 128 edges per window (static cap)
NBLK = NWIN * B       # 1862

BF16 = mybir.dt.bfloat16
F32 = mybir.dt.float32
I32 = mybir.dt.int32

_cache = {}


NQUEUE = 4


def build_program(reps: int = 1):
    nc = bacc.Bacc("TRN2", target_bir_lowering=False, debug=False,
                   num_devices=M, num_swdge_queues=NQUEUE)

    t_x = nc.dram_tensor("xbf", [N, 2 * D], BF16, kind="ExternalInput").ap()
    t_idx = nc.dram_tensor("idxm", [P, NBLK], I32, kind="ExternalInput").ap()
    t_dst = nc.dram_tensor("dstm", [P, NBLK], BF16, kind="ExternalInput").ap()
    t_nrm = nc.dram_tensor("nrmm", [P, NBLK], BF16, kind="ExternalInput").ap()
    t_iota = nc.dram_tensor("iota", [P, P], BF16, kind="ExternalInput").ap()
    t_W = nc.dram_tensor("Wt", [D, D], BF16, kind="ExternalInput").ap()
    t_bias = nc.dram_tensor("biasr", [P, D], F32, kind="ExternalInput").ap()
    t_out = nc.dram_tensor("out", [NPC, D], F32, kind="ExternalOutput").ap()

    with tile.TileContext(nc) as tc:
        with (
            tc.tile_pool(name="const", bufs=1) as constp,
            tc.tile_pool(name="meta", bufs=1) as metap,
            tc.tile_pool(name="gath", bufs=12) as gathp,
            tc.tile_pool(name="onehot", bufs=3) as onep,
            tc.tile_pool(name="sb", bufs=4) as sbp,
            tc.tile_pool(name="agg", bufs=4, space="PSUM") as aggp,
            tc.tile_pool(name="proj", bufs=2, space="PSUM") as projp,
        ):
            iota_sb = constp.tile([P, P], BF16)
            nc.sync.dma_start(out=iota_sb[:], in_=t_iota[:])
            W_sb = constp.tile([D, D], BF16)
            nc.sync.dma_start(out=W_sb[:], in_=t_W[:])
            bias_sb = constp.tile([P, D], F32)
            nc.sync.dma_start(out=bias_sb[:], in_=t_bias[:])
            idx_sb = metap.tile([P, NBLK], I32)
            nc.sync.dma_start(out=idx_sb[:], in_=t_idx[:])
            dst_sb = metap.tile([P, NBLK], BF16)
            nc.sync.dma_start(out=dst_sb[:], in_=t_dst[:])
            nrm_sb = metap.tile([P, NBLK], BF16)
            nc.sync.dma_start(out=nrm_sb[:], in_=t_nrm[:])

            for _rep in range(reps):
                for w in range(NWIN):
                    oh = onep.tile([P, B, P], BF16, tag="oh")
                    nc.vector.tensor_tensor(
                        out=oh[:],
                        in0=dst_sb[:, w * B:(w + 1) * B].unsqueeze(2)
                            .to_broadcast([P, B, P]),
                        in1=iota_sb[:].unsqueeze(1).to_broadcast([P, B, P]),
                        op=mybir.AluOpType.is_equal,
                    )
                    nc.vector.tensor_tensor(
                        out=oh[:],
                        in0=oh[:],
                        in1=nrm_sb[:, w * B:(w + 1) * B].unsqueeze(2)
                            .to_broadcast([P, B, P]),
                        op=mybir.AluOpType.mult,
                    )
                    aggT = aggp.tile([D, P], F32, space="PSUM", tag="agg")
                    for j in range(B):
                        blk = w * B + j
                        g = gathp.tile([P, D], BF16, tag="g")
                        inst = nc.gpsimd.indirect_dma_start(
                            out=g[:], out_offset=None, in_=t_x[:],
                            in_offset=bass.IndirectOffsetOnAxis(
                                ap=idx_sb[:, blk:blk + 1], axis=0))
                        if blk % NQUEUE:
                            inst.queue = f"qPoolDynamic{blk % NQUEUE}"
                        nc.tensor.matmul(
                            out=aggT[:],
                            lhsT=g[:],
                            rhs=oh[:, j, :],
                            start=(j == 0),
                            stop=(j == B - 1),
                        )
                    aggT_sb = sbp.tile([D, P], BF16, tag="aggsb")
                    nc.vector.tensor_copy(out=aggT_sb[:], in_=aggT[:])
                    pr = projp.tile([P, D], F32, space="PSUM", tag="pr")
                    nc.tensor.matmul(out=pr[:], lhsT=aggT_sb[:], rhs=W_sb[:],
                                     start=True, stop=True)
                    out_sb = sbp.tile([P, D], F32, tag="outsb")
                    nc.vector.tensor_add(out=out_sb[:], in0=pr[:],
                                         in1=bias_sb[:])
                    rows = min(P, NPC - w * P)
                    nc.sync.dma_start(out=t_out[w * P:w * P + rows, :],
                                      in_=out_sb[:rows, :])

    nc.compile()
    return nc


def _prep_inputs(x, edge_index, W, b):
    x = np.asarray(x, dtype=np.float32)
    W = np.asarray(W, dtype=np.float32)
    b = np.asarray(b, dtype=np.float32)
    ei = np.asarray(edge_index)
    src_e = ei[0].astype(np.int64)
    dst_e = ei[1].astype(np.int64)

    deg = (np.bincount(dst_e, minlength=N) + 1).astype(np.float32)
    dinv = (1.0 / np.sqrt(deg)).astype(np.float32)

    loop = np.arange(N, dtype=np.int64)
    src = np.concatenate([src_e, loop])
    dst = np.concatenate([dst_e, loop])
    norm = np.concatenate([dinv[src_e] * dinv[dst_e],
                           1.0 / deg]).astype(np.float32)

    core = dst // NPC
    ldst = dst - core * NPC
    w = ldst >> 7
    doff = (ldst & 127).astype(np.float32)

    gwin = core * NWIN + w
    order = np.argsort(gwin, kind="stable")
    gwin_s = gwin[order]
    nwin_tot = M * NWIN
    starts = np.searchsorted(gwin_s, np.arange(nwin_tot))
    ends = np.searchsorted(gwin_s, np.arange(nwin_tot), side="right")
    assert (ends - starts).max() <= B * P, f"overflow {(ends-starts).max()}"
    rank = np.empty_like(order)
    rank[order] = np.arange(len(order)) - starts[gwin_s]

    blk = w * B + (rank >> 7)          # per-core block id (window-major)
    lane = rank & 127

    bf = ml_dtypes.bfloat16
    idxm = np.zeros((M, P, NBLK), np.int32)
    dstm = np.zeros((M, P, NBLK), np.float32)
    nrmm = np.zeros((M, P, NBLK), np.float32)
    idxm[core, lane, blk] = src
    dstm[core, lane, blk] = doff
    nrmm[core, lane, blk] = norm

    x_pad = np.zeros((N, 2 * D), bf)
    x_pad[:, :D] = x.astype(bf)
    iota = np.tile(np.arange(P, dtype=np.float32), (P, 1)).astype(bf)
    W_bf = W.astype(bf)
    bias_rep = np.tile(b, (P, 1)).astype(np.float32)

    in_maps = []
    for c in range(M):
        in_maps.append({
            "xbf": x_pad,
            "idxm": idxm[c],
            "dstm": dstm[c].astype(bf),
            "nrmm": nrmm[c].astype(bf),
            "iota": iota,
            "Wt": W_bf,
            "biasr": bias_rep,
        })
    return in_maps


class SpmdRunner:
    """Cached-executable SPMD runner: jit the bass program once, reuse the
    compiled callable across calls (mirrors bass2jax.run_bass_via_pjrt's
    multi-core path, minus per-call re-jitting)."""

    def __init__(self, nc, n_cores=M):
        install_neuronx_cc_hook()
        self.nc = nc
        self.n_cores = n_cores
        assert nc.dbg_addr is None

        partition_name = (nc.partition_id_tensor.name
                          if nc.partition_id_tensor else None)
        in_names, out_names, out_avals, zero_outs = [], [], [], []
        for alloc in nc.m.functions[0].allocations:
            if not isinstance(alloc, mybir.MemoryLocationSet):
                continue
            name = alloc.memorylocations[0].name
            if alloc.kind == "ExternalInput":
                if name != partition_name:
                    in_names.append(name)
            elif alloc.kind == "ExternalOutput":
                shape = tuple(alloc.tensor_shape)
                dtype = mybir.dt.np(alloc.dtype)
                out_names.append(name)
                out_avals.append(jax.core.ShapedArray(shape, dtype))
                zero_outs.append(np.zeros(shape, dtype))
        self.in_names = list(in_names)
        self.out_names = out_names
        self.out_avals = out_avals
        self.zero_outs = zero_outs
        n_params = len(self.in_names)
        n_outs = len(out_avals)
        all_in_names = self.in_names + out_names
        if partition_name is not None:
            all_in_names.append(partition_name)

        def _body(*args):
            operands = list(args)
            if partition_name is not None:
                operands.append(partition_id_tensor())
            outs = _bass_exec_p.bind(
                *operands,
                out_avals=tuple(out_avals),
                in_names=tuple(all_in_names),
                out_names=tuple(out_names),
                lowering_input_output_aliases=(),
                sim_require_finite=True,
                sim_require_nnan=True,
                nc=nc,
            )
            return tuple(outs)

        devices = jax.devices()[:n_cores]
        assert len(devices) == n_cores
        self.mesh = Mesh(np.asarray(devices), ("core",))
        in_specs = (PartitionSpec("core"),) * (n_params + n_outs)
        out_specs = (PartitionSpec("core"),) * n_outs
        self.fn = jax.jit(shard_map(_body, mesh=self.mesh, in_specs=in_specs,
                                    out_specs=out_specs, check_rep=False),
                          keep_unused=True)
        self._dev_zeros = None

    def put_inputs(self, in_maps):
        n = self.n_cores
        concat = [np.concatenate([np.asarray(in_maps[c][name])
                                  for c in range(n)], axis=0)
                  for name in self.in_names]
        sharding = jax.sharding.NamedSharding(self.mesh, PartitionSpec("core"))
        return [jax.device_put(a, sharding) for a in concat]

    def run(self, dev_inputs):
        if self._dev_zeros is None:
            sharding = jax.sharding.NamedSharding(self.mesh,
                                                  PartitionSpec("core"))
            self._dev_zeros = [
                jax.device_put(
                    np.zeros((self.n_cores * z.shape[0], *z.shape[1:]),
                             z.dtype), sharding)
                for z in self.zero_outs]
        out = self.fn(*dev_inputs, *self._dev_zeros)
        jax.block_until_ready(out)
        return out

    def results(self, out_arrs):
        n = self.n_cores
        return [
            {name: np.asarray(out_arrs[i]).reshape(
                n, *self.out_avals[i].shape)[c]
             for i, name in enumerate(self.out_names)}
            for c in range(n)
        ]

    def __call__(self, in_maps):
        return self.results(self.run(self.put_inputs(in_maps)))


def kernel(x, edge_index, W, b):
    if "runner" not in _cache:
        _cache["runner"] = SpmdRunner(build_program(reps=1), M)
    r = _cache["runner"]
    in_maps = _prep_inputs(x, edge_index, W, b)
    res = r(in_maps)
    out = np.concatenate([rr["out"] for rr in res], axis=0)
    return out.astype(np.float32)



# revision 14
# speedup vs baseline: 1.0760x; 1.0760x over previous
"""GCN layer kernel for nn_GCNLayer_20547123544324 on 8 Trainium2 NeuronCores.

Computes a PyG-style GCNConv:
    out = D^-1/2 (A + I) D^-1/2 (x @ W) + b
       == (D^-1/2 (A + I) D^-1/2 x) @ W + b        (associativity)

Node-partitioned: 12500 dst nodes per core, 98 windows of 128 dst nodes.
The per-edge row gather is HBM-latency bound (~3.5 ns/row measured, any
SWDGE primitive), so v4 minimizes gathered rows and folds all elementwise
work off the critical path:
  - norm folding: host pre-scales x' = dinv * x so the gathered row carries
    dinv[src]; dinv[dst] is applied as a per-partition output scale. The
    scatter one-hot is then a pure is_equal (pad slots dst=1000).
  - self-loops never gathered: the window's own x' rows are streamed in one
    sequential DMA (host pre-arranged lane-major) and added into the PSUM
    accumulator with a single identity matmul per window.
  - windows sorted per core by edge count (descending); slot k's capacity =
    max-over-cores k-th sorted count, computed from the actual edge data at
    kernel() time (SPMD-safe, no overflow), with a PARTIAL last gather block
    so padding rows are not fetched. Output written in slot order, host
    un-permutes.
  - per 128-edge block: indirect-DMA row gather (one row per partition, the
    HW-supported SWDGE form), round-robin over 4 queues; per window: PE
    accumulates aggT[64,128] += g_blk.T @ onehot_blk in PSUM, projects
    through W, DVE applies dinv[dst] scale + bias.

Self-contained: hardcoded N=100000, E=1600000, D=64, 8 cores.
"""
import numpy as np
import ml_dtypes

import jax
from jax.sharding import Mesh, PartitionSpec
from jax.experimental.shard_map import shard_map

import concourse.bass as bass
import concourse.mybir as mybir
import concourse.tile as tile
from concourse import bacc
from concourse.bass2jax import _bass_exec_p, install_neuronx_cc_hook, \
    partition_id_tensor

N = 100000
E = 1600000
D = 64
M = 8                 # cores
NPC = N // M          # 12500 nodes per core
P = 128
NWIN = -(-NPC // P)   # 98 windows (last holds 84 nodes)

BF16 = mybir.dt.bfloat16
F32 = mybir.dt.float32
I32 = mybir.dt.int32

_cache = {}

NQUEUE = 4


def build_program(caps, reps: int = 1):
    """caps: per-slot edge capacities (len NWIN, max-over-cores sorted
    counts, NOT rounded to 128)."""
    caps = tuple(int(v) for v in caps)
    nblk = [-(-c // P) for c in caps]               # blocks per slot
    offs = np.concatenate([[0], np.cumsum(nblk)]).astype(int)
    totblk = int(offs[-1])

    nc = bacc.Bacc("TRN2", target_bir_lowering=False, debug=False,
                   num_devices=M, num_swdge_queues=NQUEUE)

    t_x = nc.dram_tensor("xbf", [N, 2 * D], BF16, kind="ExternalInput").ap()
    t_idx = nc.dram_tensor("idxm", [P, totblk], I32, kind="ExternalInput").ap()
    t_dst = nc.dram_tensor("dstm", [P, totblk], BF16,
                           kind="ExternalInput").ap()
    t_xloop = nc.dram_tensor("xloop", [P, NWIN * D], BF16,
                             kind="ExternalInput").ap()
    t_dinv = nc.dram_tensor("dinvw", [P, NWIN], F32, kind="ExternalInput").ap()
    t_iota = nc.dram_tensor("iota", [P, P], BF16, kind="ExternalInput").ap()
    t_ident = nc.dram_tensor("ident", [P, P], BF16, kind="ExternalInput").ap()
    t_W = nc.dram_tensor("Wt", [D, D], BF16, kind="ExternalInput").ap()
    t_bias = nc.dram_tensor("biasr", [P, D], F32, kind="ExternalInput").ap()
    t_out = nc.dram_tensor("out", [P, NWIN * D], F32,
                           kind="ExternalOutput").ap()

    with tile.TileContext(nc) as tc:
        with (
            tc.tile_pool(name="const", bufs=1) as constp,
            tc.tile_pool(name="meta", bufs=1) as metap,
            tc.tile_pool(name="gath", bufs=3) as gathp,
            tc.tile_pool(name="onehot", bufs=3) as onep,
            tc.tile_pool(name="sb", bufs=3) as sbp,
            tc.tile_pool(name="agg", bufs=4, space="PSUM") as aggp,
            tc.tile_pool(name="proj", bufs=2, space="PSUM") as projp,
        ):
            iota_sb = constp.tile([P, P], BF16)
            nc.sync.dma_start(out=iota_sb[:], in_=t_iota[:])
            ident_sb = constp.tile([P, P], BF16)
            nc.sync.dma_start(out=ident_sb[:], in_=t_ident[:])
            zero_sb = constp.tile([P, D], BF16)
            nc.vector.tensor_tensor(out=zero_sb[:], in0=iota_sb[:, :D],
                                    in1=iota_sb[:, :D],
                                    op=mybir.AluOpType.subtract)
            W_sb = constp.tile([D, D], BF16)
            nc.sync.dma_start(out=W_sb[:], in_=t_W[:])
            bias_sb = constp.tile([P, D], F32)
            nc.sync.dma_start(out=bias_sb[:], in_=t_bias[:])
            dinv_sb = constp.tile([P, NWIN], F32)
            nc.sync.dma_start(out=dinv_sb[:], in_=t_dinv[:])
            xloop_sb = constp.tile([P, NWIN * D], BF16)
            nc.sync.dma_start(out=xloop_sb[:], in_=t_xloop[:])
            idx_sb = metap.tile([P, totblk], I32)
            nc.sync.dma_start(out=idx_sb[:], in_=t_idx[:])
            dst_sb = metap.tile([P, totblk], BF16)
            nc.sync.dma_start(out=dst_sb[:], in_=t_dst[:])

            qc = 0
            for _rep in range(reps):
                for k in range(NWIN):
                    b0 = int(offs[k])
                    nb = nblk[k]
                    g = gathp.tile([P, nb * D], BF16, tag="g")
                    for j in range(nb):
                        rows = max(2, min(P, caps[k] - j * P))
                        if rows < P:
                            # ungathered lanes would hold stale bits
                            # (NaN x 0 = NaN through the PE): zero the
                            # block first, the gather overwrites [0:rows)
                            nc.vector.tensor_copy(
                                out=g[:, j * D:(j + 1) * D],
                                in_=zero_sb[:])
                        inst = nc.gpsimd.indirect_dma_start(
                            out=g[:rows, j * D:(j + 1) * D],
                            out_offset=None, in_=t_x[:],
                            in_offset=bass.IndirectOffsetOnAxis(
                                ap=idx_sb[:rows, b0 + j:b0 + j + 1], axis=0))
                        if qc % NQUEUE:
                            inst.queue = f"qPoolDynamic{qc % NQUEUE}"
                        qc += 1
                    oh = onep.tile([P, nb, P], BF16, tag="oh")
                    nc.vector.tensor_tensor(
                        out=oh[:],
                        in0=dst_sb[:, b0:b0 + nb].unsqueeze(2)
                            .to_broadcast([P, nb, P]),
                        in1=iota_sb[:].unsqueeze(1).to_broadcast([P, nb, P]),
                        op=mybir.AluOpType.is_equal,
                    )
                    aggT = aggp.tile([D, P], F32, space="PSUM", tag="agg")
                    # self-loop contribution: aggT += xloop_slot.T
                    nc.tensor.matmul(
                        out=aggT[:],
                        lhsT=xloop_sb[:, k * D:(k + 1) * D],
                        rhs=ident_sb[:],
                        start=True, stop=False)
                    for j in range(nb):
                        nc.tensor.matmul(
                            out=aggT[:],
                            lhsT=g[:, j * D:(j + 1) * D],
                            rhs=oh[:, j, :],
                            start=False,
                            stop=(j == nb - 1),
                        )
                    aggT_sb = sbp.tile([D, P], BF16, tag="aggsb")
                    nc.vector.tensor_copy(out=aggT_sb[:], in_=aggT[:])
                    pr = projp.tile([P, D], F32, space="PSUM", tag="pr")
                    nc.tensor.matmul(out=pr[:], lhsT=aggT_sb[:], rhs=W_sb[:],
                                     start=True, stop=True)
                    out_sb = sbp.tile([P, D], F32, tag="outsb")
                    nc.vector.tensor_tensor(
                        out=out_sb[:],
                        in0=pr[:],
                        in1=dinv_sb[:, k:k + 1].to_broadcast([P, D]),
                        op=mybir.AluOpType.mult,
                    )
                    nc.vector.tensor_add(out=out_sb[:], in0=out_sb[:],
                                         in1=bias_sb[:])
                    nc.sync.dma_start(out=t_out[:, k * D:(k + 1) * D],
                                      in_=out_sb[:])

    nc.compile()
    return nc


def _prep_inputs(x, edge_index, W, b):
    x = np.asarray(x, dtype=np.float32)
    W = np.asarray(W, dtype=np.float32)
    b = np.asarray(b, dtype=np.float32)
    ei = np.asarray(edge_index)
    src = ei[0].astype(np.int64)
    dst = ei[1].astype(np.int64)

    deg = (np.bincount(dst, minlength=N) + 1).astype(np.float32)
    dinv = (1.0 / np.sqrt(deg)).astype(np.float32)

    core = dst // NPC
    w = (dst - core * NPC) >> 7
    doff = ((dst - core * NPC) & 127).astype(np.float32)

    cw = core * NWIN + w
    count = np.bincount(cw, minlength=M * NWIN).reshape(M, NWIN)
    perm = np.argsort(-count, axis=1, kind="stable")      # [M, NWIN] slot->win
    slot_of = np.empty_like(perm)
    np.put_along_axis(slot_of, perm, np.arange(NWIN)[None, :], axis=1)
    sorted_cnt = np.take_along_axis(count, perm, axis=1)
    caps = sorted_cnt.max(axis=0).astype(int)             # [NWIN] exact
    caps = np.maximum(caps, 1)
    nblk = -(-caps // P)
    offs = np.concatenate([[0], np.cumsum(nblk)]).astype(int)
    totblk = int(offs[-1])

    slot = slot_of[core, w]
    gslot = core * NWIN + slot
    order = np.lexsort((src, gslot))
    gslot_s = gslot[order]
    starts = np.searchsorted(gslot_s, np.arange(M * NWIN))
    rank = np.empty(len(order), np.int64)
    rank[order] = np.arange(len(order)) - starts[gslot_s]
    assert (rank < caps[slot]).all()

    lane = rank & 127                  # block-major: sorted srcs fill a
    blk = (rank >> 7) + offs[slot]     # block's lanes consecutively

    bf = ml_dtypes.bfloat16
    idxm = np.zeros((M, P, totblk), np.int32)
    dstm = np.full((M, P, totblk), 1000.0, np.float32)
    idxm[core, lane, blk] = src
    dstm[core, lane, blk] = doff

    # lane-major self-loop rows and dinv, in slot order, per core
    node = (np.arange(M)[:, None, None] * NPC
            + perm[:, None, :] * P
            + np.arange(P)[None, :, None])                # [M, P, NWIN]
    valid = node < (np.arange(M)[:, None, None] + 1) * NPC
    nsafe = np.minimum(node, N - 1)
    dinvw = np.where(valid, dinv[nsafe], 0.0).astype(np.float32)

    xs = (x * dinv[:, None]).astype(np.float32)
    xloop = np.where(valid[..., None], xs[nsafe], 0.0)    # [M, P, NWIN, D]
    xloop = xloop.reshape(M, P, NWIN * D).astype(bf)

    x_pad = np.zeros((N, 2 * D), bf)
    x_pad[:, :D] = xs.astype(bf)
    iota = np.tile(np.arange(P, dtype=np.float32), (P, 1)).astype(bf)
    ident = np.eye(P, dtype=np.float32).astype(bf)
    W_bf = W.astype(bf)
    bias_rep = np.tile(b, (P, 1)).astype(np.float32)

    in_maps = []
    for c in range(M):
        in_maps.append({
            "xbf": x_pad,
            "idxm": idxm[c],
            "dstm": dstm[c].astype(bf),
            "xloop": xloop[c],
            "dinvw": dinvw[c],
            "iota": iota,
            "ident": ident,
            "Wt": W_bf,
            "biasr": bias_rep,
        })
    return in_maps, tuple(int(v) for v in caps), perm


class SpmdRunner:
    """Cached-executable SPMD runner: jit the bass program once, reuse the
    compiled callable across calls (mirrors bass2jax.run_bass_via_pjrt's
    multi-core path, minus per-call re-jitting)."""

    def __init__(self, nc, n_cores=M):
        install_neuronx_cc_hook()
        self.nc = nc
        self.n_cores = n_cores
        assert nc.dbg_addr is None

        partition_name = (nc.partition_id_tensor.name
                          if nc.partition_id_tensor else None)
        in_names, out_names, out_avals, zero_outs = [], [], [], []
        for alloc in nc.m.functions[0].allocations:
            if not isinstance(alloc, mybir.MemoryLocationSet):
                continue
            name = alloc.memorylocations[0].name
            if alloc.kind == "ExternalInput":
                if name != partition_name:
                    in_names.append(name)
            elif alloc.kind == "ExternalOutput":
                shape = tuple(alloc.tensor_shape)
                dtype = mybir.dt.np(alloc.dtype)
                out_names.append(name)
                out_avals.append(jax.core.ShapedArray(shape, dtype))
                zero_outs.append(np.zeros(shape, dtype))
        self.in_names = list(in_names)
        self.out_names = out_names
        self.out_avals = out_avals
        self.zero_outs = zero_outs
        n_params = len(self.in_names)
        n_outs = len(out_avals)
        all_in_names = self.in_names + out_names
        if partition_name is not None:
            all_in_names.append(partition_name)

        def _body(*args):
            operands = list(args)
            if partition_name is not None:
                operands.append(partition_id_tensor())
            outs = _bass_exec_p.bind(
                *operands,
                out_avals=tuple(out_avals),
                in_names=tuple(all_in_names),
                out_names=tuple(out_names),
                lowering_input_output_aliases=(),
                sim_require_finite=True,
                sim_require_nnan=True,
                nc=nc,
            )
            return tuple(outs)

        devices = jax.devices()[:n_cores]
        assert len(devices) == n_cores
        self.mesh = Mesh(np.asarray(devices), ("core",))
        in_specs = (PartitionSpec("core"),) * (n_params + n_outs)
        out_specs = (PartitionSpec("core"),) * n_outs
        self.fn = jax.jit(shard_map(_body, mesh=self.mesh, in_specs=in_specs,
                                    out_specs=out_specs, check_rep=False),
                          keep_unused=True)
        self._dev_zeros = None

    def put_inputs(self, in_maps):
        n = self.n_cores
        concat = [np.concatenate([np.asarray(in_maps[c][name])
                                  for c in range(n)], axis=0)
                  for name in self.in_names]
        sharding = jax.sharding.NamedSharding(self.mesh, PartitionSpec("core"))
        return [jax.device_put(a, sharding) for a in concat]

    def run(self, dev_inputs):
        if self._dev_zeros is None:
            sharding = jax.sharding.NamedSharding(self.mesh,
                                                  PartitionSpec("core"))
            self._dev_zeros = [
                jax.device_put(
                    np.zeros((self.n_cores * z.shape[0], *z.shape[1:]),
                             z.dtype), sharding)
                for z in self.zero_outs]
        out = self.fn(*dev_inputs, *self._dev_zeros)
        jax.block_until_ready(out)
        return out

    def results(self, out_arrs):
        n = self.n_cores
        return [
            {name: np.asarray(out_arrs[i]).reshape(
                n, *self.out_avals[i].shape)[c]
             for i, name in enumerate(self.out_names)}
            for c in range(n)
        ]

    def __call__(self, in_maps):
        return self.results(self.run(self.put_inputs(in_maps)))


def kernel(x, edge_index, W, b):
    in_maps, caps, perm = _prep_inputs(x, edge_index, W, b)
    key = ("runner", caps)
    if key not in _cache:
        _cache[key] = SpmdRunner(build_program(caps, reps=1), M)
    r = _cache[key]
    res = r(in_maps)

    out = np.empty((N, D), np.float32)
    for c in range(M):
        raw = res[c]["out"].reshape(P, NWIN, D)      # [lane, slot, feat]
        for k in range(NWIN):
            w0 = perm[c, k] * P
            rows = min(P, NPC - w0)
            out[c * NPC + w0: c * NPC + w0 + rows] = raw[:rows, k]
    return out.astype(np.float32)


# revision 15
# speedup vs baseline: 6.0722x; 5.6430x over previous
"""GCN layer kernel for nn_GCNLayer_20547123544324 on 8 Trainium2 NeuronCores.

Computes a PyG-style GCNConv:
    out = D^-1/2 (A + I) D^-1/2 (x @ W) + b
       == (D^-1/2 (A + I) D^-1/2 x) @ W + b        (associativity)

Node-partitioned: 12500 dst nodes per core, 98 windows of 128 dst nodes.
The per-edge row fetch is the bottleneck; the SWDGE vector-gather ucode
(dma_gather: shared int16 index list, 16 rows per DMA descriptor) moves
random rows ~3x faster per row than per-partition indirect DMA, so v5:
  - gathers via dma_gather in 1024-index chunks (the descriptor-ring limit)
    round-robin over the 4 SWDGE queues; needs the GPSIMD mlp library.
  - int16 indices only address 32768 rows, so x' is split into 4 DRAM
    subtables of 25000 rows; edges are bucketed by (window, subtable), each
    window segment padded to a 128 multiple with index-0 slots (killed by
    the one-hot) so every 128-lane block is window-pure.
  - norm folding: host pre-scales x' = dinv * x so the gathered row carries
    dinv[src]; dinv[dst] is applied as a per-partition output scale; the
    scatter one-hot is a pure is_equal (pad slots dst=1000).
  - self-loops never gathered: the window's own x' rows are streamed in one
    sequential DMA (host pre-arranged lane-major) and added into PSUM with
    one identity matmul per window.
  - capacities are data-adaptive: cap[w][t] = roundup128(max-over-cores
    count) from the actual edges at kernel() time (SPMD-safe, no overflow).
  - per window: PE accumulates aggT[64,128] += g_blk.T @ onehot_blk over
    its 4 subtable segments in PSUM, projects through W, DVE applies the
    dinv[dst] scale + bias, one batched DMA out per group.

Self-contained: hardcoded N=100000, E=1600000, D=64, 8 cores.
"""
import numpy as np
import ml_dtypes

import jax
from jax.sharding import Mesh, PartitionSpec
from jax.experimental.shard_map import shard_map

import concourse.bass as bass
import concourse.mybir as mybir
import concourse.tile as tile
from concourse import bacc
from concourse.library_config import mlp as mlp_lib
from concourse.bass2jax import _bass_exec_p, install_neuronx_cc_hook, \
    partition_id_tensor

N = 100000
E = 1600000
D = 64
M = 8                 # cores
NPC = N // M          # 12500 nodes per core
P = 128
NWIN = -(-NPC // P)   # 98 windows (last holds 84 nodes)
NSUB = 4              # subtables (int16 index range)
SUBN = N // NSUB      # 25000 rows per subtable
GWIN = 5              # windows per gather group

BF16 = mybir.dt.bfloat16
F32 = mybir.dt.float32
I16 = mybir.dt.int16

_cache = {}

NQUEUE = 4
CHUNK = 8             # blocks per dma_gather: 1024 idxs is the HW ring limit


def _layout(caps):
    """caps: [NWIN][NSUB] slot counts (multiples of 128). Returns block/idx
    column offsets in (group, subtable, window, block) order."""
    caps = np.asarray(caps, int)
    nblk = caps // P
    groups = [list(range(b, min(b + GWIN, NWIN)))
              for b in range(0, NWIN, GWIN)]
    blkoff = np.zeros((NWIN, NSUB), int)    # global block column of segment
    inst = []                               # (g, t, blk_start, nblk_total)
    run = 0
    for g, wl in enumerate(groups):
        for t in range(NSUB):
            seg0 = run
            for w in wl:
                blkoff[w, t] = run
                run += nblk[w, t]
            inst.append((g, t, seg0, run - seg0))
    return groups, nblk, blkoff, inst, run   # run = TOTBLK


def build_program(caps, reps: int = 1):
    caps = [tuple(int(v) for v in row) for row in caps]
    groups, nblk, blkoff, insts, totblk = _layout(caps)

    nc = bacc.Bacc("TRN2", target_bir_lowering=False, debug=False,
                   num_devices=M, num_swdge_queues=NQUEUE)

    t_xs = [nc.dram_tensor(f"xbf{t}", [SUBN, 2 * D], BF16,
                           kind="ExternalInput").ap() for t in range(NSUB)]
    t_idx = nc.dram_tensor("idxm", [P, totblk * 8], I16,
                           kind="ExternalInput").ap()   # 128/16 wrap cols
    t_dst = nc.dram_tensor("dstm", [P, totblk], BF16,
                           kind="ExternalInput").ap()
    t_xloop = nc.dram_tensor("xloop", [P, NWIN * D], BF16,
                             kind="ExternalInput").ap()
    t_dinv = nc.dram_tensor("dinvw", [P, NWIN], F32, kind="ExternalInput").ap()
    t_iota = nc.dram_tensor("iota", [P, P], BF16, kind="ExternalInput").ap()
    t_ident = nc.dram_tensor("ident", [P, P], BF16, kind="ExternalInput").ap()
    t_W = nc.dram_tensor("Wt", [D, D], BF16, kind="ExternalInput").ap()
    t_bias = nc.dram_tensor("biasr", [P, D], F32, kind="ExternalInput").ap()
    t_out = nc.dram_tensor("out", [P, NWIN * D], F32,
                           kind="ExternalOutput").ap()

    with tile.TileContext(nc) as tc:
        with (
            tc.tile_pool(name="const", bufs=1) as constp,
            tc.tile_pool(name="meta", bufs=1) as metap,
            tc.tile_pool(name="gath", bufs=2) as gathp,
            tc.tile_pool(name="onehot", bufs=2) as onep,
            tc.tile_pool(name="sb", bufs=3) as sbp,
            tc.tile_pool(name="agg", bufs=4, space="PSUM") as aggp,
            tc.tile_pool(name="proj", bufs=2, space="PSUM") as projp,
        ):
            nc.gpsimd.load_library(mlp_lib)
            iota_sb = constp.tile([P, P], BF16)
            nc.sync.dma_start(out=iota_sb[:], in_=t_iota[:])
            ident_sb = constp.tile([P, P], BF16)
            nc.sync.dma_start(out=ident_sb[:], in_=t_ident[:])
            W_sb = constp.tile([D, D], BF16)
            nc.sync.dma_start(out=W_sb[:], in_=t_W[:])
            bias_sb = constp.tile([P, D], F32)
            nc.sync.dma_start(out=bias_sb[:], in_=t_bias[:])
            dinv_sb = constp.tile([P, NWIN], F32)
            nc.sync.dma_start(out=dinv_sb[:], in_=t_dinv[:])
            xloop_sb = constp.tile([P, NWIN * D], BF16)
            nc.sync.dma_start(out=xloop_sb[:], in_=t_xloop[:])
            idx_sb = metap.tile([P, totblk * 8], I16)
            nc.sync.dma_start(out=idx_sb[:], in_=t_idx[:])
            dst_sb = metap.tile([P, totblk], BF16)
            nc.sync.dma_start(out=dst_sb[:], in_=t_dst[:])

            qc = 0
            for _rep in range(reps):
                for g, wl in enumerate(groups):
                    gt = {}
                    ot = {}
                    for (gg, t, b0, nb) in insts:
                        if gg != g or nb == 0:
                            continue
                        gtile = gathp.tile([P, nb, P], BF16, tag=f"g{t}")
                        for cb0 in range(0, nb, CHUNK):
                            cb = min(CHUNK, nb - cb0)
                            nc.gpsimd.dma_gather(
                                out_ap=gtile[:, cb0:cb0 + cb, :],
                                in_ap=t_xs[t][:],
                                idxs_ap=idx_sb[:, (b0 + cb0) * 8:
                                               (b0 + cb0 + cb) * 8],
                                num_idxs=cb * P, num_idxs_reg=cb * P,
                                elem_size=2 * D, queue_num=qc % NQUEUE)
                            qc += 1
                        ohtile = onep.tile([P, nb, P], BF16, tag=f"oh{t}")
                        nc.vector.tensor_tensor(
                            out=ohtile[:],
                            in0=dst_sb[:, b0:b0 + nb].unsqueeze(2)
                                .to_broadcast([P, nb, P]),
                            in1=iota_sb[:].unsqueeze(1)
                                .to_broadcast([P, nb, P]),
                            op=mybir.AluOpType.is_equal,
                        )
                        gt[t] = (gtile, b0)
                        ot[t] = ohtile

                    out_sb = sbp.tile([P, len(wl) * D], F32, tag="outsb")
                    for ki, w in enumerate(wl):
                        segs = [(t, blkoff[w, t] - gt[t][1], nblk[w, t])
                                for t in range(NSUB)
                                if nblk[w, t] > 0 and t in gt]
                        nb_tot = sum(s[2] for s in segs)
                        aggT = aggp.tile([D, P], F32, space="PSUM", tag="agg")
                        # self-loop contribution: aggT += xloop_w.T
                        nc.tensor.matmul(
                            out=aggT[:],
                            lhsT=xloop_sb[:, w * D:(w + 1) * D],
                            rhs=ident_sb[:],
                            start=True, stop=(nb_tot == 0))
                        done = 0
                        for (t, lb, nbw) in segs:
                            gtile = gt[t][0]
                            ohtile = ot[t]
                            for j in range(nbw):
                                nc.tensor.matmul(
                                    out=aggT[:],
                                    lhsT=gtile[:, lb + j, 0:D],
                                    rhs=ohtile[:, lb + j, :],
                                    start=False,
                                    stop=(done == nb_tot - 1),
                                )
                                done += 1
                        aggT_sb = sbp.tile([D, P], BF16, tag="aggsb")
                        nc.vector.tensor_copy(out=aggT_sb[:], in_=aggT[:])
                        pr = projp.tile([P, D], F32, space="PSUM", tag="pr")
                        nc.tensor.matmul(out=pr[:], lhsT=aggT_sb[:],
                                         rhs=W_sb[:], start=True, stop=True)
                        nc.vector.tensor_tensor(
                            out=out_sb[:, ki * D:(ki + 1) * D],
                            in0=pr[:],
                            in1=dinv_sb[:, w:w + 1].to_broadcast([P, D]),
                            op=mybir.AluOpType.mult,
                        )
                        nc.vector.tensor_add(
                            out=out_sb[:, ki * D:(ki + 1) * D],
                            in0=out_sb[:, ki * D:(ki + 1) * D],
                            in1=bias_sb[:])
                    nc.sync.dma_start(
                        out=t_out[:, wl[0] * D:(wl[-1] + 1) * D],
                        in_=out_sb[:, :len(wl) * D])

    nc.compile()
    return nc


def _prep_inputs(x, edge_index, W, b):
    x = np.asarray(x, dtype=np.float32)
    W = np.asarray(W, dtype=np.float32)
    b = np.asarray(b, dtype=np.float32)
    ei = np.asarray(edge_index)
    src = ei[0].astype(np.int64)
    dst = ei[1].astype(np.int64)

    deg = (np.bincount(dst, minlength=N) + 1).astype(np.float32)
    dinv = (1.0 / np.sqrt(deg)).astype(np.float32)

    core = dst // NPC
    w = (dst - core * NPC) >> 7
    doff = ((dst - core * NPC) & 127).astype(np.float32)
    t = src // SUBN
    lidx = (src - t * SUBN).astype(np.int16)

    # counts per (core, window, subtable) -> shared caps (multiples of 128)
    key = (core * NWIN + w) * NSUB + t
    cnt = np.bincount(key, minlength=M * NWIN * NSUB).reshape(M, NWIN, NSUB)
    caps = (-(-cnt.max(axis=0) // P) * P).astype(int)     # [NWIN, NSUB]
    groups, nblk, blkoff, insts, totblk = _layout(caps)

    # rank within (core, window, subtable), ordered by src for locality
    order = np.lexsort((src, key))
    key_s = key[order]
    starts = np.searchsorted(key_s, np.arange(M * NWIN * NSUB))
    rank = np.empty(len(order), np.int64)
    rank[order] = np.arange(len(order)) - starts[key_s]
    assert (rank < caps[w, t]).all()

    lane = (rank & 127).astype(np.int64)   # block-major per dma_gather
    blk = (rank >> 7) + blkoff[w, t]

    bf = ml_dtypes.bfloat16
    idxw_c = np.zeros((M, P, totblk * 8), np.int16)
    dstm_c = np.full((M, P, totblk), 1000.0, np.float32)
    for c in range(M):
        mask = core == c
        fl = np.zeros(totblk * P, np.int16)                # pad -> row 0
        fl[blk[mask] * P + lane[mask]] = lidx[mask]
        idxw_c[c] = np.tile(fl.reshape(-1, 16).T, (8, 1))
        dm = np.full((P, totblk), 1000.0, np.float32)
        dm[lane[mask], blk[mask]] = doff[mask]
        dstm_c[c] = dm

    # lane-major self-loop rows and dinv per core (window order, no perm)
    node = (np.arange(M)[:, None, None] * NPC
            + np.arange(NWIN)[None, None, :] * P
            + np.arange(P)[None, :, None])                 # [M, P, NWIN]
    valid = node < (np.arange(M)[:, None, None] + 1) * NPC
    nsafe = np.minimum(node, N - 1)
    dinvw = np.where(valid, dinv[nsafe], 0.0).astype(np.float32)

    xs = (x * dinv[:, None]).astype(np.float32)
    xloop = np.where(valid[..., None], xs[nsafe], 0.0)     # [M, P, NWIN, D]
    xloop = xloop.transpose(0, 1, 2, 3).reshape(M, P, NWIN * D).astype(bf)

    x_pad = np.zeros((N, 2 * D), bf)
    x_pad[:, :D] = xs.astype(bf)
    iota = np.tile(np.arange(P, dtype=np.float32), (P, 1)).astype(bf)
    ident = np.eye(P, dtype=np.float32).astype(bf)
    W_bf = W.astype(bf)
    bias_rep = np.tile(b, (P, 1)).astype(np.float32)

    in_maps = []
    for c in range(M):
        im = {
            "idxm": idxw_c[c],
            "dstm": dstm_c[c].astype(bf),
            "xloop": xloop[c],
            "dinvw": dinvw[c],
            "iota": iota,
            "ident": ident,
            "Wt": W_bf,
            "biasr": bias_rep,
        }
        for tt in range(NSUB):
            im[f"xbf{tt}"] = x_pad[tt * SUBN:(tt + 1) * SUBN]
        in_maps.append(im)
    return in_maps, tuple(tuple(int(v) for v in row) for row in caps), None


class SpmdRunner:
    """Cached-executable SPMD runner: jit the bass program once, reuse the
    compiled callable across calls (mirrors bass2jax.run_bass_via_pjrt's
    multi-core path, minus per-call re-jitting)."""

    def __init__(self, nc, n_cores=M):
        install_neuronx_cc_hook()
        self.nc = nc
        self.n_cores = n_cores
        assert nc.dbg_addr is None

        partition_name = (nc.partition_id_tensor.name
                          if nc.partition_id_tensor else None)
        in_names, out_names, out_avals, zero_outs = [], [], [], []
        for alloc in nc.m.functions[0].allocations:
            if not isinstance(alloc, mybir.MemoryLocationSet):
                continue
            name = alloc.memorylocations[0].name
            if alloc.kind == "ExternalInput":
                if name != partition_name:
                    in_names.append(name)
            elif alloc.kind == "ExternalOutput":
                shape = tuple(alloc.tensor_shape)
                dtype = mybir.dt.np(alloc.dtype)
                out_names.append(name)
                out_avals.append(jax.core.ShapedArray(shape, dtype))
                zero_outs.append(np.zeros(shape, dtype))
        self.in_names = list(in_names)
        self.out_names = out_names
        self.out_avals = out_avals
        self.zero_outs = zero_outs
        n_params = len(self.in_names)
        n_outs = len(out_avals)
        all_in_names = self.in_names + out_names
        if partition_name is not None:
            all_in_names.append(partition_name)

        def _body(*args):
            operands = list(args)
            if partition_name is not None:
                operands.append(partition_id_tensor())
            outs = _bass_exec_p.bind(
                *operands,
                out_avals=tuple(out_avals),
                in_names=tuple(all_in_names),
                out_names=tuple(out_names),
                lowering_input_output_aliases=(),
                sim_require_finite=True,
                sim_require_nnan=True,
                nc=nc,
            )
            return tuple(outs)

        devices = jax.devices()[:n_cores]
        assert len(devices) == n_cores
        self.mesh = Mesh(np.asarray(devices), ("core",))
        in_specs = (PartitionSpec("core"),) * (n_params + n_outs)
        out_specs = (PartitionSpec("core"),) * n_outs
        self.fn = jax.jit(shard_map(_body, mesh=self.mesh, in_specs=in_specs,
                                    out_specs=out_specs, check_rep=False),
                          keep_unused=True)
        self._dev_zeros = None

    def put_inputs(self, in_maps):
        n = self.n_cores
        concat = [np.concatenate([np.asarray(in_maps[c][name])
                                  for c in range(n)], axis=0)
                  for name in self.in_names]
        sharding = jax.sharding.NamedSharding(self.mesh, PartitionSpec("core"))
        return [jax.device_put(a, sharding) for a in concat]

    def run(self, dev_inputs):
        if self._dev_zeros is None:
            sharding = jax.sharding.NamedSharding(self.mesh,
                                                  PartitionSpec("core"))
            self._dev_zeros = [
                jax.device_put(
                    np.zeros((self.n_cores * z.shape[0], *z.shape[1:]),
                             z.dtype), sharding)
                for z in self.zero_outs]
        out = self.fn(*dev_inputs, *self._dev_zeros)
        jax.block_until_ready(out)
        return out

    def results(self, out_arrs):
        n = self.n_cores
        return [
            {name: np.asarray(out_arrs[i]).reshape(
                n, *self.out_avals[i].shape)[c]
             for i, name in enumerate(self.out_names)}
            for c in range(n)
        ]

    def __call__(self, in_maps):
        return self.results(self.run(self.put_inputs(in_maps)))


def kernel(x, edge_index, W, b):
    in_maps, caps, _ = _prep_inputs(x, edge_index, W, b)
    key = ("runner", caps)
    if key not in _cache:
        _cache[key] = SpmdRunner(build_program(caps, reps=1), M)
    r = _cache[key]
    res = r(in_maps)

    out = np.empty((N, D), np.float32)
    for c in range(M):
        raw = res[c]["out"].reshape(P, NWIN, D)      # [lane, window, feat]
        full = raw.transpose(1, 0, 2).reshape(NWIN * P, D)
        out[c * NPC:(c + 1) * NPC] = full[:NPC]
    return out.astype(np.float32)


# revision 16
# speedup vs baseline: 9.9985x; 1.6466x over previous
"""GCN layer kernel for nn_GCNLayer_20547123544324 on 8 Trainium2 NeuronCores.

Computes a PyG-style GCNConv:
    out = D^-1/2 (A + I) D^-1/2 (x @ W) + b
       == (D^-1/2 (A + I) D^-1/2 x) @ W + b        (associativity)

Node-partitioned: 12500 dst nodes per core, 98 windows of 128 dst nodes.
The per-edge row fetch is the bottleneck; the SWDGE vector-gather ucode
(dma_gather: shared int16 index list, 16 rows per DMA descriptor) moves
random rows ~3x faster per row than per-partition indirect DMA, so v5:
  - gathers via dma_gather in 512-index chunks (~4 in flight per ring)
    round-robin over the 4 SWDGE queues; needs the GPSIMD mlp library.
  - int16 indices only address 32768 rows, so x' is split into 4 DRAM
    subtables of 25000 rows; edges are bucketed by (window, subtable), each
    window segment padded to a 128 multiple with index-0 slots (killed by
    the one-hot) so every 128-lane block is window-pure.
  - norm folding: host pre-scales x' = dinv * x so the gathered row carries
    dinv[src]; dinv[dst] is applied as a per-partition output scale; the
    scatter one-hot is a pure is_equal (pad slots dst=1000).
  - self-loops never gathered: the window's own x' rows are streamed in one
    sequential DMA (host pre-arranged lane-major) and added into PSUM with
    one identity matmul per window.
  - capacities are data-adaptive: cap[w][t] = roundup128(max-over-cores
    count) from the actual edges at kernel() time (SPMD-safe, no overflow).
  - per window: PE accumulates aggT[64,128] += g_blk.T @ onehot_blk over
    its 4 subtable segments in PSUM, projects through W, DVE applies the
    dinv[dst] scale + bias, one batched DMA out per group.

Self-contained: hardcoded N=100000, E=1600000, D=64, 8 cores.
"""
import numpy as np
import ml_dtypes

import jax
from jax.sharding import Mesh, PartitionSpec
from jax.experimental.shard_map import shard_map

import concourse.bass as bass
import concourse.mybir as mybir
import concourse.tile as tile
from concourse import bacc
from concourse.library_config import mlp as mlp_lib
from concourse.bass2jax import _bass_exec_p, install_neuronx_cc_hook, \
    partition_id_tensor

N = 100000
E = 1600000
D = 64
M = 8                 # cores
NPC = N // M          # 12500 nodes per core
P = 128
NWIN = -(-NPC // P)   # 98 windows (last holds 84 nodes)
NSUB = 4              # subtables (int16 index range)
SUBN = N // NSUB      # 25000 rows per subtable
GWIN = 5              # windows per gather group

BF16 = mybir.dt.bfloat16
F32 = mybir.dt.float32
I16 = mybir.dt.int16

_cache = {}

NQUEUE = 4
CHUNK = 8             # blocks per dma_gather: 1024 idxs is the HW ring limit


def _layout(caps):
    """caps: [NWIN][NSUB] slot counts (multiples of 128). Returns block/idx
    column offsets in (group, subtable, window, block) order."""
    caps = np.asarray(caps, int)
    nblk = caps // P
    groups = [list(range(b, min(b + GWIN, NWIN)))
              for b in range(0, NWIN, GWIN)]
    blkoff = np.zeros((NWIN, NSUB), int)    # global block column of segment
    inst = []                               # (g, t, blk_start, nblk_total)
    run = 0
    for g, wl in enumerate(groups):
        for t in range(NSUB):
            seg0 = run
            for w in wl:
                blkoff[w, t] = run
                run += nblk[w, t]
            inst.append((g, t, seg0, run - seg0))
    return groups, nblk, blkoff, inst, run   # run = TOTBLK


def build_program(caps, reps: int = 1):
    caps = [tuple(int(v) for v in row) for row in caps]
    groups, nblk, blkoff, insts, totblk = _layout(caps)

    nc = bacc.Bacc("TRN2", target_bir_lowering=False, debug=False,
                   num_devices=M, num_swdge_queues=NQUEUE)

    t_xs = [nc.dram_tensor(f"xbf{t}", [SUBN, 2 * D], BF16,
                           kind="ExternalInput").ap() for t in range(NSUB)]
    t_idx = nc.dram_tensor("idxm", [P, totblk * 8], I16,
                           kind="ExternalInput").ap()   # 128/16 wrap cols
    t_dst = nc.dram_tensor("dstm", [P, totblk], BF16,
                           kind="ExternalInput").ap()
    t_xloop = nc.dram_tensor("xloop", [P, NWIN * D], BF16,
                             kind="ExternalInput").ap()
    t_dinv = nc.dram_tensor("dinvw", [P, NWIN], F32, kind="ExternalInput").ap()
    t_iota = nc.dram_tensor("iota", [P, P], BF16, kind="ExternalInput").ap()
    t_ident = nc.dram_tensor("ident", [P, P], BF16, kind="ExternalInput").ap()
    t_W = nc.dram_tensor("Wt", [D, D], BF16, kind="ExternalInput").ap()
    t_bias = nc.dram_tensor("biasr", [P, D], F32, kind="ExternalInput").ap()
    t_out = nc.dram_tensor("out", [P, NWIN * D], F32,
                           kind="ExternalOutput").ap()

    with tile.TileContext(nc) as tc:
        with (
            tc.tile_pool(name="const", bufs=1) as constp,
            tc.tile_pool(name="meta", bufs=1) as metap,
            tc.tile_pool(name="gath", bufs=2) as gathp,
            tc.tile_pool(name="onehot", bufs=2) as onep,
            tc.tile_pool(name="sb", bufs=3) as sbp,
            tc.tile_pool(name="agg", bufs=4, space="PSUM") as aggp,
            tc.tile_pool(name="proj", bufs=2, space="PSUM") as projp,
        ):
            nc.gpsimd.load_library(mlp_lib)
            iota_sb = constp.tile([P, P], BF16)
            nc.sync.dma_start(out=iota_sb[:], in_=t_iota[:])
            ident_sb = constp.tile([P, P], BF16)
            nc.sync.dma_start(out=ident_sb[:], in_=t_ident[:])
            W_sb = constp.tile([D, D], BF16)
            nc.sync.dma_start(out=W_sb[:], in_=t_W[:])
            bias_sb = constp.tile([P, D], F32)
            nc.sync.dma_start(out=bias_sb[:], in_=t_bias[:])
            dinv_sb = constp.tile([P, NWIN], F32)
            nc.sync.dma_start(out=dinv_sb[:], in_=t_dinv[:])
            xloop_sb = constp.tile([P, NWIN * D], BF16)
            nc.sync.dma_start(out=xloop_sb[:], in_=t_xloop[:])
            idx_sb = metap.tile([P, totblk * 8], I16)
            nc.sync.dma_start(out=idx_sb[:], in_=t_idx[:])
            dst_sb = metap.tile([P, totblk], BF16)
            nc.sync.dma_start(out=dst_sb[:], in_=t_dst[:])

            qc = 0
            for _rep in range(reps):
                for g, wl in enumerate(groups):
                    gt = {}
                    ot = {}
                    for (gg, t, b0, nb) in insts:
                        if gg != g or nb == 0:
                            continue
                        gtile = gathp.tile([P, nb, P], BF16, tag=f"g{t}")
                        for cb0 in range(0, nb, CHUNK):
                            cb = min(CHUNK, nb - cb0)
                            nc.gpsimd.dma_gather(
                                out_ap=gtile[:, cb0:cb0 + cb, :],
                                in_ap=t_xs[t][:],
                                idxs_ap=idx_sb[:, (b0 + cb0) * 8:
                                               (b0 + cb0 + cb) * 8],
                                num_idxs=cb * P, num_idxs_reg=cb * P,
                                elem_size=2 * D, queue_num=qc % NQUEUE)
                            qc += 1
                        ohtile = onep.tile([P, nb, P], BF16, tag=f"oh{t}")
                        nc.vector.tensor_tensor(
                            out=ohtile[:],
                            in0=dst_sb[:, b0:b0 + nb].unsqueeze(2)
                                .to_broadcast([P, nb, P]),
                            in1=iota_sb[:].unsqueeze(1)
                                .to_broadcast([P, nb, P]),
                            op=mybir.AluOpType.is_equal,
                        )
                        gt[t] = (gtile, b0)
                        ot[t] = ohtile

                    out_sb = sbp.tile([P, len(wl) * D], F32, tag="outsb")
                    for ki, w in enumerate(wl):
                        segs = [(t, blkoff[w, t] - gt[t][1], nblk[w, t])
                                for t in range(NSUB)
                                if nblk[w, t] > 0 and t in gt]
                        nb_tot = sum(s[2] for s in segs)
                        aggT = aggp.tile([D, P], F32, space="PSUM", tag="agg")
                        # self-loop contribution: aggT += xloop_w.T
                        nc.tensor.matmul(
                            out=aggT[:],
                            lhsT=xloop_sb[:, w * D:(w + 1) * D],
                            rhs=ident_sb[:],
                            start=True, stop=(nb_tot == 0))
                        done = 0
                        for (t, lb, nbw) in segs:
                            gtile = gt[t][0]
                            ohtile = ot[t]
                            for j in range(nbw):
                                nc.tensor.matmul(
                                    out=aggT[:],
                                    lhsT=gtile[:, lb + j, 0:D],
                                    rhs=ohtile[:, lb + j, :],
                                    start=False,
                                    stop=(done == nb_tot - 1),
                                )
                                done += 1
                        aggT_sb = sbp.tile([D, P], BF16, tag="aggsb")
                        nc.vector.tensor_copy(out=aggT_sb[:], in_=aggT[:])
                        pr = projp.tile([P, D], F32, space="PSUM", tag="pr")
                        nc.tensor.matmul(out=pr[:], lhsT=aggT_sb[:],
                                         rhs=W_sb[:], start=True, stop=True)
                        nc.vector.tensor_tensor(
                            out=out_sb[:, ki * D:(ki + 1) * D],
                            in0=pr[:],
                            in1=dinv_sb[:, w:w + 1].to_broadcast([P, D]),
                            op=mybir.AluOpType.mult,
                        )
                        nc.vector.tensor_add(
                            out=out_sb[:, ki * D:(ki + 1) * D],
                            in0=out_sb[:, ki * D:(ki + 1) * D],
                            in1=bias_sb[:])
                    nc.sync.dma_start(
                        out=t_out[:, wl[0] * D:(wl[-1] + 1) * D],
                        in_=out_sb[:, :len(wl) * D])

    nc.compile()
    return nc


def _prep_inputs(x, edge_index, W, b):
    x = np.asarray(x, dtype=np.float32)
    W = np.asarray(W, dtype=np.float32)
    b = np.asarray(b, dtype=np.float32)
    ei = np.asarray(edge_index)
    src = ei[0].astype(np.int64)
    dst = ei[1].astype(np.int64)

    deg = (np.bincount(dst, minlength=N) + 1).astype(np.float32)
    dinv = (1.0 / np.sqrt(deg)).astype(np.float32)

    core = dst // NPC
    w = (dst - core * NPC) >> 7
    doff = ((dst - core * NPC) & 127).astype(np.float32)
    t = src // SUBN
    lidx = (src - t * SUBN).astype(np.int16)

    # counts per (core, window, subtable) -> shared caps (multiples of 128)
    key = (core * NWIN + w) * NSUB + t
    cnt = np.bincount(key, minlength=M * NWIN * NSUB).reshape(M, NWIN, NSUB)
    caps = (-(-cnt.max(axis=0) // P) * P).astype(int)     # [NWIN, NSUB]
    groups, nblk, blkoff, insts, totblk = _layout(caps)

    # rank within (core, window, subtable), ordered by src for locality
    order = np.lexsort((src, key))
    key_s = key[order]
    starts = np.searchsorted(key_s, np.arange(M * NWIN * NSUB))
    rank = np.empty(len(order), np.int64)
    rank[order] = np.arange(len(order)) - starts[key_s]
    assert (rank < caps[w, t]).all()

    lane = (rank & 127).astype(np.int64)   # block-major per dma_gather
    blk = (rank >> 7) + blkoff[w, t]

    bf = ml_dtypes.bfloat16
    idxw_c = np.zeros((M, P, totblk * 8), np.int16)
    dstm_c = np.full((M, P, totblk), 1000.0, np.float32)
    for c in range(M):
        mask = core == c
        fl = np.zeros(totblk * P, np.int16)                # pad -> row 0
        fl[blk[mask] * P + lane[mask]] = lidx[mask]
        idxw_c[c] = np.tile(fl.reshape(-1, 16).T, (8, 1))
        dm = np.full((P, totblk), 1000.0, np.float32)
        dm[lane[mask], blk[mask]] = doff[mask]
        dstm_c[c] = dm

    # lane-major self-loop rows and dinv per core (window order, no perm)
    node = (np.arange(M)[:, None, None] * NPC
            + np.arange(NWIN)[None, None, :] * P
            + np.arange(P)[None, :, None])                 # [M, P, NWIN]
    valid = node < (np.arange(M)[:, None, None] + 1) * NPC
    nsafe = np.minimum(node, N - 1)
    dinvw = np.where(valid, dinv[nsafe], 0.0).astype(np.float32)

    xs = (x * dinv[:, None]).astype(np.float32)
    xloop = np.where(valid[..., None], xs[nsafe], 0.0)     # [M, P, NWIN, D]
    xloop = xloop.transpose(0, 1, 2, 3).reshape(M, P, NWIN * D).astype(bf)

    x_pad = np.zeros((N, 2 * D), bf)
    x_pad[:, :D] = xs.astype(bf)
    iota = np.tile(np.arange(P, dtype=np.float32), (P, 1)).astype(bf)
    ident = np.eye(P, dtype=np.float32).astype(bf)
    W_bf = W.astype(bf)
    bias_rep = np.tile(b, (P, 1)).astype(np.float32)

    in_maps = []
    for c in range(M):
        im = {
            "idxm": idxw_c[c],
            "dstm": dstm_c[c].astype(bf),
            "xloop": xloop[c],
            "dinvw": dinvw[c],
            "iota": iota,
            "ident": ident,
            "Wt": W_bf,
            "biasr": bias_rep,
        }
        for tt in range(NSUB):
            im[f"xbf{tt}"] = x_pad[tt * SUBN:(tt + 1) * SUBN]
        in_maps.append(im)
    return in_maps, tuple(tuple(int(v) for v in row) for row in caps), None


class SpmdRunner:
    """Cached-executable SPMD runner: jit the bass program once, reuse the
    compiled callable across calls (mirrors bass2jax.run_bass_via_pjrt's
    multi-core path, minus per-call re-jitting)."""

    def __init__(self, nc, n_cores=M):
        install_neuronx_cc_hook()
        self.nc = nc
        self.n_cores = n_cores
        assert nc.dbg_addr is None

        partition_name = (nc.partition_id_tensor.name
                          if nc.partition_id_tensor else None)
        in_names, out_names, out_avals, zero_outs = [], [], [], []
        for alloc in nc.m.functions[0].allocations:
            if not isinstance(alloc, mybir.MemoryLocationSet):
                continue
            name = alloc.memorylocations[0].name
            if alloc.kind == "ExternalInput":
                if name != partition_name:
                    in_names.append(name)
            elif alloc.kind == "ExternalOutput":
                shape = tuple(alloc.tensor_shape)
                dtype = mybir.dt.np(alloc.dtype)
                out_names.append(name)
                out_avals.append(jax.core.ShapedArray(shape, dtype))
                zero_outs.append(np.zeros(shape, dtype))
        self.in_names = list(in_names)
        self.out_names = out_names
        self.out_avals = out_avals
        self.zero_outs = zero_outs
        n_params = len(self.in_names)
        n_outs = len(out_avals)
        all_in_names = self.in_names + out_names
        if partition_name is not None:
            all_in_names.append(partition_name)

        def _body(*args):
            operands = list(args)
            if partition_name is not None:
                operands.append(partition_id_tensor())
            outs = _bass_exec_p.bind(
                *operands,
                out_avals=tuple(out_avals),
                in_names=tuple(all_in_names),
                out_names=tuple(out_names),
                lowering_input_output_aliases=(),
                sim_require_finite=True,
                sim_require_nnan=True,
                nc=nc,
            )
            return tuple(outs)

        devices = jax.devices()[:n_cores]
        assert len(devices) == n_cores
        self.mesh = Mesh(np.asarray(devices), ("core",))
        in_specs = (PartitionSpec("core"),) * (n_params + n_outs)
        out_specs = (PartitionSpec("core"),) * n_outs
        self.fn = jax.jit(shard_map(_body, mesh=self.mesh, in_specs=in_specs,
                                    out_specs=out_specs, check_rep=False),
                          keep_unused=True)
        self._dev_zeros = None

    def put_inputs(self, in_maps):
        n = self.n_cores
        concat = [np.concatenate([np.asarray(in_maps[c][name])
                                  for c in range(n)], axis=0)
                  for name in self.in_names]
        sharding = jax.sharding.NamedSharding(self.mesh, PartitionSpec("core"))
        return [jax.device_put(a, sharding) for a in concat]

    def run(self, dev_inputs):
        if self._dev_zeros is None:
            sharding = jax.sharding.NamedSharding(self.mesh,
                                                  PartitionSpec("core"))
            self._dev_zeros = [
                jax.device_put(
                    np.zeros((self.n_cores * z.shape[0], *z.shape[1:]),
                             z.dtype), sharding)
                for z in self.zero_outs]
        out = self.fn(*dev_inputs, *self._dev_zeros)
        jax.block_until_ready(out)
        return out

    def results(self, out_arrs):
        n = self.n_cores
        return [
            {name: np.asarray(out_arrs[i]).reshape(
                n, *self.out_avals[i].shape)[c]
             for i, name in enumerate(self.out_names)}
            for c in range(n)
        ]

    def __call__(self, in_maps):
        return self.results(self.run(self.put_inputs(in_maps)))


def kernel(x, edge_index, W, b):
    in_maps, caps, _ = _prep_inputs(x, edge_index, W, b)
    key = ("runner", caps)
    if key not in _cache:
        _cache[key] = SpmdRunner(build_program(caps, reps=1), M)
    r = _cache[key]
    res = r(in_maps)

    out = np.empty((N, D), np.float32)
    for c in range(M):
        raw = res[c]["out"].reshape(P, NWIN, D)      # [lane, window, feat]
        full = raw.transpose(1, 0, 2).reshape(NWIN * P, D)
        out[c * NPC:(c + 1) * NPC] = full[:NPC]
    return out.astype(np.float32)


# revision 17
# speedup vs baseline: 83.2894x; 8.3302x over previous
"""GCN layer kernel for nn_GCNLayer_20547123544324 on 8 Trainium2 NeuronCores.

Computes a PyG-style GCNConv:
    out = D^-1/2 (A + I) D^-1/2 (x @ W) + b
       == (D^-1/2 (A + I) D^-1/2 x) @ W + b        (associativity)

Node-partitioned: 12500 dst nodes per core, 98 windows of 128 dst nodes.
The per-edge row fetch is the bottleneck; the SWDGE vector-gather ucode
(dma_gather: shared int16 index list, 16 rows per DMA descriptor) moves
random rows ~3x faster per row than per-partition indirect DMA, so v5:
  - gathers via dma_gather in 512-index chunks (~4 in flight per ring)
    round-robin over the 4 SWDGE queues; needs the GPSIMD mlp library.
  - int16 indices only address 32768 rows, so x' is split into 4 DRAM
    subtables of 25000 rows; edges are bucketed by (window, subtable), each
    window segment padded to a 128 multiple with index-0 slots (killed by
    the one-hot) so every 128-lane block is window-pure.
  - norm folding: host pre-scales x' = dinv * x so the gathered row carries
    dinv[src]; dinv[dst] is applied as a per-partition output scale; the
    scatter one-hot is a pure is_equal (pad slots dst=1000).
  - self-loops never gathered: the window's own x' rows are streamed in one
    sequential DMA (host pre-arranged lane-major) and added into PSUM with
    one identity matmul per window.
  - capacities are data-adaptive: cap[w][t] = roundup128(max-over-cores
    count) from the actual edges at kernel() time (SPMD-safe, no overflow).
  - per window: PE accumulates aggT[64,128] += g_blk.T @ onehot_blk over
    its 4 subtable segments in PSUM, projects through W, DVE applies the
    dinv[dst] scale + bias, one batched DMA out per group.

Self-contained: hardcoded N=100000, E=1600000, D=64, 8 cores.
"""
import numpy as np
import ml_dtypes

import jax
from jax.sharding import Mesh, PartitionSpec
from jax.experimental.shard_map import shard_map

import concourse.bass as bass
import concourse.mybir as mybir
import concourse.tile as tile
from concourse import bacc
from concourse.library_config import mlp as mlp_lib
from concourse.bass2jax import _bass_exec_p, install_neuronx_cc_hook, \
    partition_id_tensor

N = 100000
E = 1600000
D = 64
M = 8                 # cores
NPC = N // M          # 12500 nodes per core
P = 128
NWIN = -(-NPC // P)   # 98 windows (last holds 84 nodes)
NSUB = 4              # subtables (int16 index range)
SUBN = N // NSUB      # 25000 rows per subtable
GWIN = 3              # windows per gather group (smaller tiles -> 3-deep buffers)

BF16 = mybir.dt.bfloat16
F32 = mybir.dt.float32
I16 = mybir.dt.int16

_cache = {}

NQUEUE = 4
CHUNK = 8             # blocks per dma_gather: 1024 idxs is the HW ring limit


def _layout(caps):
    """caps: [NWIN][NSUB] slot counts (multiples of 128). Returns block/idx
    column offsets in (group, subtable, window, block) order."""
    caps = np.asarray(caps, int)
    nblk = caps // P
    groups = [list(range(b, min(b + GWIN, NWIN)))
              for b in range(0, NWIN, GWIN)]
    blkoff = np.zeros((NWIN, NSUB), int)    # global block column of segment
    inst = []                               # (g, t, blk_start, nblk_total)
    run = 0
    for g, wl in enumerate(groups):
        for t in range(NSUB):
            seg0 = run
            for w in wl:
                blkoff[w, t] = run
                run += nblk[w, t]
            inst.append((g, t, seg0, run - seg0))
    return groups, nblk, blkoff, inst, run   # run = TOTBLK


def build_program(caps, reps: int = 1):
    caps = [tuple(int(v) for v in row) for row in caps]
    groups, nblk, blkoff, insts, totblk = _layout(caps)

    nc = bacc.Bacc("TRN2", target_bir_lowering=False, debug=False,
                   num_devices=M, num_swdge_queues=NQUEUE)

    t_xs = [nc.dram_tensor(f"xbf{t}", [SUBN, 2 * D], BF16,
                           kind="ExternalInput").ap() for t in range(NSUB)]
    t_idx = nc.dram_tensor("idxm", [P, totblk * 8], I16,
                           kind="ExternalInput").ap()   # 128/16 wrap cols
    t_dst = nc.dram_tensor("dstm", [P, totblk], BF16,
                           kind="ExternalInput").ap()
    t_xloop = nc.dram_tensor("xloop", [P, NWIN * D], BF16,
                             kind="ExternalInput").ap()
    t_dinv = nc.dram_tensor("dinvw", [P, NWIN], F32, kind="ExternalInput").ap()
    t_iota = nc.dram_tensor("iota", [P, P], BF16, kind="ExternalInput").ap()
    t_ident = nc.dram_tensor("ident", [P, P], BF16, kind="ExternalInput").ap()
    t_W = nc.dram_tensor("Wt", [D, D], BF16, kind="ExternalInput").ap()
    t_bias = nc.dram_tensor("biasr", [P, D], F32, kind="ExternalInput").ap()
    t_out = nc.dram_tensor("out", [P, NWIN * D], F32,
                           kind="ExternalOutput").ap()

    with tile.TileContext(nc) as tc:
        with (
            tc.tile_pool(name="const", bufs=1) as constp,
            tc.tile_pool(name="meta", bufs=1) as metap,
            tc.tile_pool(name="gath", bufs=3) as gathp,
            tc.tile_pool(name="onehot", bufs=3) as onep,
            tc.tile_pool(name="sb", bufs=3) as sbp,
            tc.tile_pool(name="agg", bufs=4, space="PSUM") as aggp,
            tc.tile_pool(name="proj", bufs=2, space="PSUM") as projp,
        ):
            nc.gpsimd.load_library(mlp_lib)
            iota_sb = constp.tile([P, P], BF16)
            nc.sync.dma_start(out=iota_sb[:], in_=t_iota[:])
            ident_sb = constp.tile([P, P], BF16)
            nc.sync.dma_start(out=ident_sb[:], in_=t_ident[:])
            W_sb = constp.tile([D, D], BF16)
            nc.sync.dma_start(out=W_sb[:], in_=t_W[:])
            bias_sb = constp.tile([P, D], F32)
            nc.sync.dma_start(out=bias_sb[:], in_=t_bias[:])
            dinv_sb = constp.tile([P, NWIN], F32)
            nc.sync.dma_start(out=dinv_sb[:], in_=t_dinv[:])
            xloop_sb = constp.tile([P, NWIN * D], BF16)
            nc.sync.dma_start(out=xloop_sb[:], in_=t_xloop[:])
            idx_sb = metap.tile([P, totblk * 8], I16)
            nc.sync.dma_start(out=idx_sb[:], in_=t_idx[:])
            dst_sb = metap.tile([P, totblk], BF16)
            nc.sync.dma_start(out=dst_sb[:], in_=t_dst[:])

            qc = 0
            for _rep in range(reps):
                for g, wl in enumerate(groups):
                    gt = {}
                    ot = {}
                    for (gg, t, b0, nb) in insts:
                        if gg != g or nb == 0:
                            continue
                        gtile = gathp.tile([P, nb, P], BF16, tag=f"g{t}")
                        for cb0 in range(0, nb, CHUNK):
                            cb = min(CHUNK, nb - cb0)
                            nc.gpsimd.dma_gather(
                                out_ap=gtile[:, cb0:cb0 + cb, :],
                                in_ap=t_xs[t][:],
                                idxs_ap=idx_sb[:, (b0 + cb0) * 8:
                                               (b0 + cb0 + cb) * 8],
                                num_idxs=cb * P, num_idxs_reg=cb * P,
                                elem_size=2 * D, queue_num=qc % NQUEUE)
                            qc += 1
                        ohtile = onep.tile([P, nb, P], BF16, tag=f"oh{t}")
                        nc.vector.tensor_tensor(
                            out=ohtile[:],
                            in0=dst_sb[:, b0:b0 + nb].unsqueeze(2)
                                .to_broadcast([P, nb, P]),
                            in1=iota_sb[:].unsqueeze(1)
                                .to_broadcast([P, nb, P]),
                            op=mybir.AluOpType.is_equal,
                        )
                        gt[t] = (gtile, b0)
                        ot[t] = ohtile

                    out_sb = sbp.tile([P, len(wl) * D], F32, tag="outsb")
                    for ki, w in enumerate(wl):
                        segs = [(t, blkoff[w, t] - gt[t][1], nblk[w, t])
                                for t in range(NSUB)
                                if nblk[w, t] > 0 and t in gt]
                        nb_tot = sum(s[2] for s in segs)
                        aggT = aggp.tile([D, P], F32, space="PSUM", tag="agg")
                        # self-loop contribution: aggT += xloop_w.T
                        nc.tensor.matmul(
                            out=aggT[:],
                            lhsT=xloop_sb[:, w * D:(w + 1) * D],
                            rhs=ident_sb[:],
                            start=True, stop=(nb_tot == 0))
                        done = 0
                        for (t, lb, nbw) in segs:
                            gtile = gt[t][0]
                            ohtile = ot[t]
                            for j in range(nbw):
                                nc.tensor.matmul(
                                    out=aggT[:],
                                    lhsT=gtile[:, lb + j, 0:D],
                                    rhs=ohtile[:, lb + j, :],
                                    start=False,
                                    stop=(done == nb_tot - 1),
                                )
                                done += 1
                        aggT_sb = sbp.tile([D, P], BF16, tag="aggsb")
                        nc.vector.tensor_copy(out=aggT_sb[:], in_=aggT[:])
                        pr = projp.tile([P, D], F32, space="PSUM", tag="pr")
                        nc.tensor.matmul(out=pr[:], lhsT=aggT_sb[:],
                                         rhs=W_sb[:], start=True, stop=True)
                        nc.vector.tensor_tensor(
                            out=out_sb[:, ki * D:(ki + 1) * D],
                            in0=pr[:],
                            in1=dinv_sb[:, w:w + 1].to_broadcast([P, D]),
                            op=mybir.AluOpType.mult,
                        )
                        nc.vector.tensor_add(
                            out=out_sb[:, ki * D:(ki + 1) * D],
                            in0=out_sb[:, ki * D:(ki + 1) * D],
                            in1=bias_sb[:])
                    nc.sync.dma_start(
                        out=t_out[:, wl[0] * D:(wl[-1] + 1) * D],
                        in_=out_sb[:, :len(wl) * D])

    nc.compile()
    return nc


def _prep_inputs(x, edge_index, W, b):
    x = np.asarray(x, dtype=np.float32)
    W = np.asarray(W, dtype=np.float32)
    b = np.asarray(b, dtype=np.float32)
    ei = np.asarray(edge_index)
    src = ei[0].astype(np.int64)
    dst = ei[1].astype(np.int64)

    deg = (np.bincount(dst, minlength=N) + 1).astype(np.float32)
    dinv = (1.0 / np.sqrt(deg)).astype(np.float32)

    core = dst // NPC
    w = (dst - core * NPC) >> 7
    doff = ((dst - core * NPC) & 127).astype(np.float32)
    t = src // SUBN
    lidx = (src - t * SUBN).astype(np.int16)

    # counts per (core, window, subtable) -> shared caps (multiples of 128)
    key = (core * NWIN + w) * NSUB + t
    cnt = np.bincount(key, minlength=M * NWIN * NSUB).reshape(M, NWIN, NSUB)
    caps = (-(-cnt.max(axis=0) // P) * P).astype(int)     # [NWIN, NSUB]
    groups, nblk, blkoff, insts, totblk = _layout(caps)

    # rank within (core, window, subtable), ordered by src for locality
    order = np.lexsort((src, key))
    key_s = key[order]
    starts = np.searchsorted(key_s, np.arange(M * NWIN * NSUB))
    rank = np.empty(len(order), np.int64)
    rank[order] = np.arange(len(order)) - starts[key_s]
    assert (rank < caps[w, t]).all()

    lane = (rank & 127).astype(np.int64)   # block-major per dma_gather
    blk = (rank >> 7) + blkoff[w, t]

    bf = ml_dtypes.bfloat16
    idxw_c = np.zeros((M, P, totblk * 8), np.int16)
    dstm_c = np.full((M, P, totblk), 1000.0, np.float32)
    for c in range(M):
        mask = core == c
        fl = np.zeros(totblk * P, np.int16)                # pad -> row 0
        fl[blk[mask] * P + lane[mask]] = lidx[mask]
        idxw_c[c] = np.tile(fl.reshape(-1, 16).T, (8, 1))
        dm = np.full((P, totblk), 1000.0, np.float32)
        dm[lane[mask], blk[mask]] = doff[mask]
        dstm_c[c] = dm

    # lane-major self-loop rows and dinv per core (window order, no perm)
    node = (np.arange(M)[:, None, None] * NPC
            + np.arange(NWIN)[None, None, :] * P
            + np.arange(P)[None, :, None])                 # [M, P, NWIN]
    valid = node < (np.arange(M)[:, None, None] + 1) * NPC
    nsafe = np.minimum(node, N - 1)
    dinvw = np.where(valid, dinv[nsafe], 0.0).astype(np.float32)

    xs = (x * dinv[:, None]).astype(np.float32)
    xloop = np.where(valid[..., None], xs[nsafe], 0.0)     # [M, P, NWIN, D]
    xloop = xloop.transpose(0, 1, 2, 3).reshape(M, P, NWIN * D).astype(bf)

    x_pad = np.zeros((N, 2 * D), bf)
    x_pad[:, :D] = xs.astype(bf)
    iota = np.tile(np.arange(P, dtype=np.float32), (P, 1)).astype(bf)
    ident = np.eye(P, dtype=np.float32).astype(bf)
    W_bf = W.astype(bf)
    bias_rep = np.tile(b, (P, 1)).astype(np.float32)

    in_maps = []
    for c in range(M):
        im = {
            "idxm": idxw_c[c],
            "dstm": dstm_c[c].astype(bf),
            "xloop": xloop[c],
            "dinvw": dinvw[c],
            "iota": iota,
            "ident": ident,
            "Wt": W_bf,
            "biasr": bias_rep,
        }
        for tt in range(NSUB):
            im[f"xbf{tt}"] = x_pad[tt * SUBN:(tt + 1) * SUBN]
        in_maps.append(im)
    return in_maps, tuple(tuple(int(v) for v in row) for row in caps), None


class SpmdRunner:
    """Cached-executable SPMD runner: jit the bass program once, reuse the
    compiled callable across calls (mirrors bass2jax.run_bass_via_pjrt's
    multi-core path, minus per-call re-jitting)."""

    def __init__(self, nc, n_cores=M):
        install_neuronx_cc_hook()
        self.nc = nc
        self.n_cores = n_cores
        assert nc.dbg_addr is None

        partition_name = (nc.partition_id_tensor.name
                          if nc.partition_id_tensor else None)
        in_names, out_names, out_avals, zero_outs = [], [], [], []
        for alloc in nc.m.functions[0].allocations:
            if not isinstance(alloc, mybir.MemoryLocationSet):
                continue
            name = alloc.memorylocations[0].name
            if alloc.kind == "ExternalInput":
                if name != partition_name:
                    in_names.append(name)
            elif alloc.kind == "ExternalOutput":
                shape = tuple(alloc.tensor_shape)
                dtype = mybir.dt.np(alloc.dtype)
                out_names.append(name)
                out_avals.append(jax.core.ShapedArray(shape, dtype))
                zero_outs.append(np.zeros(shape, dtype))
        self.in_names = list(in_names)
        self.out_names = out_names
        self.out_avals = out_avals
        self.zero_outs = zero_outs
        n_params = len(self.in_names)
        n_outs = len(out_avals)
        all_in_names = self.in_names + out_names
        if partition_name is not None:
            all_in_names.append(partition_name)

        def _body(*args):
            operands = list(args)
            if partition_name is not None:
                operands.append(partition_id_tensor())
            outs = _bass_exec_p.bind(
                *operands,
                out_avals=tuple(out_avals),
                in_names=tuple(all_in_names),
                out_names=tuple(out_names),
                lowering_input_output_aliases=(),
                sim_require_finite=True,
                sim_require_nnan=True,
                nc=nc,
            )
            return tuple(outs)

        devices = jax.devices()[:n_cores]
        assert len(devices) == n_cores
        self.mesh = Mesh(np.asarray(devices), ("core",))
        in_specs = (PartitionSpec("core"),) * (n_params + n_outs)
        out_specs = (PartitionSpec("core"),) * n_outs
        self.fn = jax.jit(shard_map(_body, mesh=self.mesh, in_specs=in_specs,
                                    out_specs=out_specs, check_rep=False),
                          keep_unused=True)
        self._dev_zeros = None

    def put_inputs(self, in_maps):
        n = self.n_cores
        concat = [np.concatenate([np.asarray(in_maps[c][name])
                                  for c in range(n)], axis=0)
                  for name in self.in_names]
        sharding = jax.sharding.NamedSharding(self.mesh, PartitionSpec("core"))
        return [jax.device_put(a, sharding) for a in concat]

    def run(self, dev_inputs):
        if self._dev_zeros is None:
            sharding = jax.sharding.NamedSharding(self.mesh,
                                                  PartitionSpec("core"))
            self._dev_zeros = [
                jax.device_put(
                    np.zeros((self.n_cores * z.shape[0], *z.shape[1:]),
                             z.dtype), sharding)
                for z in self.zero_outs]
        out = self.fn(*dev_inputs, *self._dev_zeros)
        jax.block_until_ready(out)
        return out

    def results(self, out_arrs):
        n = self.n_cores
        return [
            {name: np.asarray(out_arrs[i]).reshape(
                n, *self.out_avals[i].shape)[c]
             for i, name in enumerate(self.out_names)}
            for c in range(n)
        ]

    def __call__(self, in_maps):
        return self.results(self.run(self.put_inputs(in_maps)))


def kernel(x, edge_index, W, b):
    in_maps, caps, _ = _prep_inputs(x, edge_index, W, b)
    key = ("runner", caps)
    if key not in _cache:
        _cache[key] = SpmdRunner(build_program(caps, reps=1), M)
    r = _cache[key]
    res = r(in_maps)

    out = np.empty((N, D), np.float32)
    for c in range(M):
        raw = res[c]["out"].reshape(P, NWIN, D)      # [lane, window, feat]
        full = raw.transpose(1, 0, 2).reshape(NWIN * P, D)
        out[c * NPC:(c + 1) * NPC] = full[:NPC]
    return out.astype(np.float32)
